# revision 58
# baseline (speedup 1.0000x reference)
"""Trainium2 Bass kernel for nn_ACNN (attention CNN with attentive pooling).

Data-parallel over the batch dim: 128 batches -> 16 per NeuronCore x 8 cores.
Each core runs an identical single-core graph on its shard; no collectives.

Per-batch pipeline (all matmuls on TensorE, N=512 free dim):
  xct   = x_concat^T (d on partitions) via PE transposes
  A1/A2 = x @ (We@e) accumulated for all 16 batches into one (16,512) PSUM
          via block-diagonal lhsT (full fp32: softmax over ~N(0,256) logits
          amplifies matmul rounding, bf16-grade precision is NOT enough here)
  alpha = 0.5*(softmax(A1)+softmax(A2))  (free-axis softmax, fused exp+sum)
  conv  = tanh(alpha[l] * windowed-conv(xc))   window shifts are free-axis AP
          offsets into the zero-padded xct; alpha applied as per-partition
          ACT scale on PSUM eviction (float32r matmuls)
  G     = conv @ (U @ rel_w^T)      E = exp(G)   (G in [-14,14]: no max sub)
  s     = ones^T @ E (column sums), AP = E/s folded into the wo eviction
  wo    = relu(max_r (E^T @ conv)[r,f] / s[r])
"""

import sys

sys.path.insert(0, "/opt/trn_rl_repo")

import numpy as np

import concourse.bass as bass
import concourse.bacc as bacc
import concourse.mybir as mybir
import concourse.tile as tile
from concourse.bass_utils import run_bass_kernel_spmd
from concourse.masks import make_identity

B, L, DW, DP, NF = 128, 512, 256, 64, 512
D = DW + 2 * DP            # 384
NCORES = 8
BS = B // NCORES           # 16 batches per core
F32 = mybir.dt.float32
F32R = mybir.dt.float32r
BF16 = mybir.dt.bfloat16
FP16 = mybir.dt.float16
AF = mybir.ActivationFunctionType
ALU = mybir.AluOpType
AX = mybir.AxisListType


def r32(ap):
    return ap.bitcast(F32R)


def _build_graph(has_bias: bool):
    nc = bacc.Bacc("TRN2", target_bir_lowering=False, debug=False)

    x_ext = nc.dram_tensor("x", [BS, L, DW], F32, kind="ExternalInput").ap()
    e1_ext = nc.dram_tensor("e1", [BS, DW], F32, kind="ExternalInput").ap()
    e2_ext = nc.dram_tensor("e2", [BS, DW], F32, kind="ExternalInput").ap()
    pv_ext = nc.dram_tensor("posVec", [BS, L, 2 * DP], F32, kind="ExternalInput").ap()
    We1_ext = nc.dram_tensor("We1", [DW, DW], F32, kind="ExternalInput").ap()
    We2_ext = nc.dram_tensor("We2", [DW, DW], F32, kind="ExternalInput").ap()
    U_ext = nc.dram_tensor("U", [NF, NF], F32, kind="ExternalInput").ap()
    cw_ext = nc.dram_tensor("conv_w", [NF, 3 * D], F32, kind="ExternalInput").ap()
    cb_ext = nc.dram_tensor("conv_b", [NF], F32, kind="ExternalInput").ap()
    rw_ext = nc.dram_tensor("rel_w", [NF, NF], F32, kind="ExternalInput").ap()
    out_ext = nc.dram_tensor("out", [BS, NF], F32, kind="ExternalOutput").ap()

    with tile.TileContext(nc) as tc:
        _body(nc, tc, has_bias,
              x_ext, e1_ext, e2_ext, pv_ext, We1_ext, We2_ext, U_ext,
              cw_ext, cb_ext, rw_ext, out_ext)
    nc.compile()
    return nc


def _body(nc, tc, has_bias, x_ext, e1_ext, e2_ext, pv_ext, We1_ext, We2_ext,
          U_ext, cw_ext, cb_ext, rw_ext, out_ext):
    from contextlib import ExitStack

    GB = 4           # batches per pipeline group
    NG = BS // GB
    ctx = ExitStack()
    const = ctx.enter_context(tc.tile_pool(name="const", bufs=1))
    ps_big = ctx.enter_context(tc.tile_pool(name="ps_big", bufs=4, space="PSUM"))
    ps_bf = ctx.enter_context(tc.tile_pool(name="ps_bf", bufs=2, space="PSUM"))
    ps_sm = ctx.enter_context(tc.tile_pool(name="ps_sm", bufs=1, space="PSUM"))

    def bigps(name, shape=(128, 512), dtype=F32):
        return ps_big.tile(list(shape), dtype, tag="big", name=name)

    # ---- constants ----
    ident = const.tile([128, 128], F32, tag="ident", name="ident")
    make_identity(nc, ident[:])
    ident_bf = const.tile([128, 128], BF16, tag="ident_bf", name="ident_bf")
    nc.vector.tensor_copy(out=ident_bf[:], in_=ident[:])
    ones_bf = const.tile([128, 1], BF16, tag="ones_bf", name="ones_bf")
    nc.gpsimd.memset(ones_bf[:], 1.0)
    ones_r = const.tile([128, 1], F32R, tag="ones_r", name="ones_r")
    nc.vector.tensor_copy(out=ones_r[:], in_=ones_bf[:])
    ident_r = const.tile([128, 128], F32R, tag="ident_r", name="ident_r")
    nc.vector.tensor_copy(out=ident_r[:], in_=ident[:])
    # mask[p, b, b'] = 1 if b == b' else 0  (for block-diagonal V build)
    mask = const.tile([128, BS, BS], F32, tag="mask", name="mask")
    nc.gpsimd.memset(mask[:], 0.0)
    nc.gpsimd.affine_select(
        out=mask[:], in_=mask[:], compare_op=ALU.not_equal, fill=1.0,
        base=0, pattern=[[1, BS], [-1, BS]], channel_multiplier=0)

    convwT = const.tile([128, 9, 512], BF16, tag="convwT", name="convwT")
    M_sb = const.tile([128, 4, 512], BF16, tag="M_sb", name="M_sb")
    # The A-path stays full fp32: softmax over ~N(0,256) logits amplifies
    # input rounding (bf16 -> ~0.5 max-abs output error, fp16 -> ~4e-2);
    # fp32 keeps max-abs error at ~5e-3. V1 and V2 are stacked along the
    # output-row (M) axis so one N=512 stream computes both A1 and A2.
    # dims: [p, b, dc, group, v, within-group] so the per-(b,dc,group) lhsT
    # slice is one contiguous 2*GB-wide free range (matmul weights APs must
    # flatten to a single free dim)
    VblkC = const.tile([128, BS, 2, NG, 2, GB], F32, tag="VblkC", name="VblkC")
    alpha_colT = const.tile([128, 4, BS], F32, tag="alphaT", name="alphaT")
    out_acc = const.tile([128, BS, 4], F32, tag="out_acc", name="out_acc")
    outT = const.tile([BS, NF], F32, tag="outT", name="outT")
    if has_bias:
        b_bcast = const.tile([128, 512], F32, tag="b_bcast", name="b_bcast")

    # ---- setup: transpose weights, precompute M = U @ rel_w^T, V1/V2 ----
    # DMA issues go smallest-first so the PE's first work (the We/e
    # transposes) needs only ~0.5MB of loads instead of waiting ~12us for
    # the 2.25MB conv_w tensor; conv_w isn't consumed until phase B.
    with tc.tile_pool(name="setup", bufs=1) as setup:
        st_ws, st_es = {}, {}
        for name, W_ext, e_ext in (("1", We1_ext, e1_ext), ("2", We2_ext, e2_ext)):
            st_w = setup.tile([128, 2, 256], F32, tag=f"st_w{name}", name=f"st_w{name}")
            nc.sync.dma_start(out=st_w[:], in_=W_ext.rearrange("(dc p) e -> p dc e", p=128))
            st_e = setup.tile([BS, 256], F32, tag=f"st_e{name}", name=f"st_e{name}")
            nc.sync.dma_start(out=st_e[:], in_=e_ext)
            st_ws[name], st_es[name] = st_w, st_e
        st_u = setup.tile([128, 4, 512], F32, tag="st_u", name="st_u")
        nc.sync.dma_start(out=st_u[:], in_=U_ext.rearrange("(fc p) t -> p fc t", p=128))
        st_rw = setup.tile([128, 4, 512], F32, tag="st_rw", name="st_rw")
        nc.sync.dma_start(out=st_rw[:], in_=rw_ext.rearrange("(rc p) t -> p rc t", p=128))
        st_cw = setup.tile([128, 4, 3 * D], F32, tag="st_cw", name="st_cw")
        nc.sync.dma_start(out=st_cw[:], in_=cw_ext.rearrange("(fc p) kd -> p fc kd", p=128))

        # We1/We2 (256d, 256e) -> WeT (e part, d free); e1/e2 -> eT (e part, b free)
        V_sb = {}
        for name in ("1", "2"):
            st_w, st_e = st_ws[name], st_es[name]
            WT = setup.tile([128, 2, 256], F32, tag=f"WT{name}", name=f"WT{name}")
            for ec in range(2):
                psw = bigps(f"ps_WT{name}_{ec}", (128, 256))
                for dc in range(2):
                    nc.tensor.matmul(psw[:, dc * 128:(dc + 1) * 128],
                                     st_w[:, dc, ec * 128:(ec + 1) * 128], ident[:],
                                     is_transpose=True, start=(dc == 0), stop=(dc == 1))
                nc.vector.tensor_copy(out=WT[:, ec, :], in_=psw[:])
            pse = ps_sm.tile([128, 2 * BS], F32, tag="sm", name=f"ps_eT{name}")
            for ec in range(2):
                nc.tensor.matmul(pse[:, ec * BS:(ec + 1) * BS],
                                 st_e[:, ec * 128:(ec + 1) * 128], ident[:BS, :BS],
                                 is_transpose=True, start=(ec == 0), stop=(ec == 1))
            eT = setup.tile([128, 2, BS], F32, tag=f"eT{name}", name=f"eT{name}")
            nc.vector.tensor_copy(out=eT[:], in_=pse[:])
            # V[d, b] = sum_e We[d, e] e[b, e]   (full fp32 - feeds the A softmax)
            V = setup.tile([128, 2, BS], F32, tag=f"V{name}", name=f"V{name}")
            for dc in range(2):
                psv = ps_sm.tile([128, BS], F32, tag="sm", name=f"ps_V{name}_{dc}")
                for ec in range(2):
                    nc.tensor.matmul(psv[:], WT[:, ec, dc * 128:(dc + 1) * 128],
                                     eT[:, ec, :], start=(ec == 0), stop=(ec == 1))
                nc.vector.tensor_copy(out=V[:, dc, :], in_=psv[:])
            V_sb[name] = V

        # VblkC[p, b, dc, q, v, g] = V_v[p, dc, b] * (b == q*GB+g)
        # (one TT op per (v, group): the ISA allows at most 3 free dims)
        for v, name in ((0, "1"), (1, "2")):
            for q in range(NG):
                nc.vector.tensor_tensor(
                    VblkC[:, :, :, q, v, :],
                    V_sb[name].rearrange("p dc b -> p b dc")[:, :, :, None]
                    .to_broadcast([128, BS, 2, GB]),
                    mask[:, :, None, q * GB:(q + 1) * GB]
                    .to_broadcast([128, BS, 2, GB]),
                    ALU.mult)

        # U (512f, 512t) -> UT (t part, f free); rel_w (512r, 512t) -> relwT
        UT = setup.tile([128, 4, 512], F32R, tag="UT", name="UT")
        relwT = setup.tile([128, 4, 512], F32R, tag="relwT", name="relwT")
        for tc_i in range(4):
            psu = bigps(f"ps_UT_{tc_i}")
            for fc in range(4):
                nc.tensor.matmul(psu[:, fc * 128:(fc + 1) * 128],
                                 st_u[:, fc, tc_i * 128:(tc_i + 1) * 128], ident[:],
                                 is_transpose=True, start=(fc == 0), stop=(fc == 3))
            nc.vector.tensor_copy(out=UT[:, tc_i, :], in_=psu[:])
            psr = bigps(f"ps_rwT_{tc_i}")
            for rc in range(4):
                nc.tensor.matmul(psr[:, rc * 128:(rc + 1) * 128],
                                 st_rw[:, rc, tc_i * 128:(tc_i + 1) * 128], ident[:],
                                 is_transpose=True, start=(rc == 0), stop=(rc == 3))
            nc.vector.tensor_copy(out=relwT[:, tc_i, :], in_=psr[:])
        # M[f, r] = sum_t U[f,t] rel_w[r,t]
        for fc in range(4):
            psm = bigps(f"ps_M_{fc}")
            for tc_i in range(4):
                nc.tensor.matmul(psm[:], UT[:, tc_i, fc * 128:(fc + 1) * 128],
                                 relwT[:, tc_i, :],
                                 start=(tc_i == 0), stop=(tc_i == 3))
            nc.vector.tensor_copy(out=M_sb[:, fc, :], in_=psm[:])

        # conv_w (512f, 1152kd) -> convwT[kd-part chunks, f free]
        # (last: its 2.25MB load is the slowest, and phase B is its consumer)
        for kd in range(9):
            ps = bigps(f"ps_cwT_{kd}")
            for fc in range(4):
                nc.tensor.matmul(ps[:, fc * 128:(fc + 1) * 128],
                                 st_cw[:, fc, kd * 128:(kd + 1) * 128], ident[:],
                                 is_transpose=True, start=(fc == 0), stop=(fc == 3))
            nc.vector.tensor_copy(out=convwT[:, kd, :], in_=ps[:])

        if has_bias:
            st_b = setup.tile([1, 512], F32, tag="st_b", name="st_b")
            nc.sync.dma_start(out=st_b[:], in_=cb_ext[None, :])
            ones_row = setup.tile([1, 128], F32, tag="ones_row", name="ones_row")
            nc.gpsimd.memset(ones_row[:], 1.0)
            psb = bigps("ps_bb")
            nc.tensor.matmul(psb[:], ones_row[:], st_b[:], start=True, stop=True)
            nc.vector.tensor_copy(out=b_bcast[:], in_=psb[:])

    # ---- main pools ----
    xct_pool = ctx.enter_context(tc.tile_pool(name="xct", bufs=BS))
    work = ctx.enter_context(tc.tile_pool(name="work", bufs=3))
    wo_pool = ctx.enter_context(tc.tile_pool(name="wo", bufs=2))
    stage = ctx.enter_context(tc.tile_pool(name="stage", bufs=3))

    # Batches are processed in NG groups of GB. Group q's loads/transposes/
    # attention logits (phase A) are emitted interleaved with group q-1's
    # conv/pool work (phase B) so the PE never drains while DMA/DVE chains
    # mature. alpha (the batch-softmax barrier) is per group.
    xcts = [None] * BS
    A_pss = {}
    pending_A = {}

    def phaseA_batch(b, q, i):
        # f32r transposes run at 1.5 cyc/row vs fp32's 2.0 (storage is
        # identical 4-byte; rel-err gate verifies transpose-mode does not
        # round the data on HW)
        x_st = stage.tile([128, 4, DW], F32R, tag="x_st", name=f"x_st_{b}")
        nc.sync.dma_start(out=x_st[:], in_=x_ext[b].rearrange("(lc p) d -> p lc d", p=128).bitcast(F32R))
        pv_st = stage.tile([128, 4, 2 * DP], F32R, tag="pv_st", name=f"pv_st_{b}")
        nc.sync.dma_start(out=pv_st[:], in_=pv_ext[b].rearrange("(lc p) d -> p lc d", p=128).bitcast(F32R))
        xct = xct_pool.tile([128, 3, L + 2], BF16, tag="xct", name=f"xct_{b}")
        nc.gpsimd.memset(xct[:, :, 0:1], 0.0)
        nc.gpsimd.memset(xct[:, :, L + 1:L + 2], 0.0)
        A_ps = A_pss[q]
        xtmps = []
        for dd in range(3):
            ps = bigps(f"ps_xT_{b}_{dd}", dtype=F32R)
            for lc in range(4):
                src = (x_st[:, lc, dd * 128:(dd + 1) * 128] if dd < 2
                       else pv_st[:, lc, :])
                nc.tensor.matmul(ps[:, lc * 128:(lc + 1) * 128], src, ident_r[:],
                                 is_transpose=True, start=(lc == 0), stop=(lc == 3))
            nc.vector.tensor_copy(out=xct[:, dd, 1:L + 1], in_=ps[:])
            if dd < 2:
                # full-fp32 copy for the A-path (bf16 rounding of x would
                # perturb the ~N(0,256) attention logits too much)
                xtmp = stage.tile([128, 512], F32, tag="xtmp", bufs=4,
                                  name=f"xtmp_{b}_{dd}")
                nc.scalar.activation(out=xtmp[:], in_=ps[:], func=AF.Copy)
                xtmps.append(xtmp)
        xcts[b] = xct
        pending_A[(q, i)] = xtmps

    def phaseA_mm(q, i):
        # Deferred one interleave step so the xtmp ACT copies (queued behind
        # the previous batch's tanh/exp on the in-order Scalar engine) have
        # matured before the in-order PE reaches these matmuls. One matmul
        # computes both A1 (rows 0..GB-1) and A2 (rows GB..2GB-1).
        b = q * GB + i
        xtmps = pending_A.pop((q, i))
        A_ps = A_pss[q]
        for dd in range(2):
            nc.tensor.matmul(A_ps[:], VblkC[:, b, dd, q, :, :],
                             xtmps[dd][:], start=(i == 0 and dd == 0),
                             stop=(i == GB - 1 and dd == 1))

    def alpha_group(q):
        # softmax over both stacked logit halves in one pass; the
        # alpha = 0.5*(softmax(A1)+softmax(A2)) combine happens after the
        # transpose, where the halves sit GB columns apart (a free-axis
        # add -- DVE cannot add across partitions).
        A_ps = A_pss.pop(q)
        neg_m = work.tile([2 * GB, 1], F32, tag="neg_m", name=f"neg_m_{q}")
        nc.vector.reduce_max(out=neg_m[:], in_=A_ps[:], axis=AX.X, negate=True)
        E_row = work.tile([2 * GB, 512], F32, tag="E_row", name=f"E_row_{q}")
        s_row = work.tile([2 * GB, 1], F32, tag="s_row", name=f"s_row_{q}")
        nc.scalar.activation(out=E_row[:], in_=A_ps[:], func=AF.Exp,
                             bias=neg_m[:], scale=1.0, accum_out=s_row[:])
        r_row = work.tile([2 * GB, 1], F32, tag="r_row", name=f"r_row_{q}")
        nc.vector.reciprocal(out=r_row[:], in_=s_row[:])
        nc.vector.tensor_scalar_mul(r_row[:], r_row[:], 0.5)
        nc.vector.tensor_scalar_mul(E_row[:], E_row[:], r_row[:])
        psa = ps_sm.tile([128, 4, 2 * GB], F32, tag="sm", name=f"ps_alphaT_{q}")
        for lc in range(4):
            nc.tensor.matmul(psa[:, lc, :], E_row[:, lc * 128:(lc + 1) * 128],
                             ident[:2 * GB, :2 * GB],
                             is_transpose=True, start=(lc == 0), stop=(lc == 3))
        # TT ops may read only one PSUM operand: evict, then add the halves
        aT = work.tile([128, 4, 2 * GB], F32, tag="aT", name=f"aT_{q}")
        nc.vector.tensor_copy(out=aT[:], in_=psa[:])
        nc.vector.tensor_tensor(
            alpha_colT[:, :, q * GB:(q + 1) * GB],
            aT[:, :, 0:GB], aT[:, :, GB:2 * GB], ALU.add)

    def alloc_A(q):
        # bufs=1: group q's tile is read+released by alpha(q) before group
        # q+1's first A-matmul executes, so one slot never conflicts
        A_pss[q] = ps_sm.tile([2 * GB, 512], F32, tag="Aps", bufs=1,
                              name=f"A_ps_{q}")

    # ---- phase B: conv -> G -> softmax-pool -> wo ----
    # The per-batch tail (max over r + transpose + reduce) is a serial
    # DVE chain; emit it one batch late so the PE fills the wait with the
    # next batch's conv matmuls (also keeps HAM un-throttled).
    woTs = {}

    def head(b):
        xct = xcts[b]
        convLF = work.tile([128, 4, 512], BF16, tag="convLF", name=f"convLF_{b}")
        for lc in range(4):
            psc = bigps(f"ps_conv_{b}_{lc}")
            first = True
            for k in range(3):
                for dc in range(3):
                    nc.tensor.matmul(
                        psc[:],
                        xct[:, dc, lc * 128 + k: lc * 128 + k + 128],
                        convwT[:, k * 3 + dc, :],
                        start=first, stop=(k == 2 and dc == 2))
                    first = False
            if has_bias:
                t_sc = work.tile([128, 512], F32, tag="t_sc", name=f"t_sc_{b}_{lc}")
                nc.vector.tensor_scalar_mul(t_sc[:], psc[:], alpha_colT[:, lc, b:b + 1])
                nc.vector.tensor_tensor(t_sc[:], t_sc[:], b_bcast[:], ALU.add)
                nc.scalar.activation(out=convLF[:, lc, :], in_=t_sc[:], func=AF.Tanh)
            else:
                nc.scalar.activation(out=convLF[:, lc, :], in_=psc[:], func=AF.Tanh,
                                     bias=0.0, scale=alpha_colT[:, lc, b:b + 1])
        convFL = work.tile([128, 4, 512], BF16, tag="convFL", name=f"convFL_{b}")
        for fc in range(4):
            pst = ps_bf.tile([128, 512], BF16, tag="bigbf", name=f"ps_cT_{b}_{fc}")
            for lc in range(4):
                nc.tensor.matmul(pst[:, lc * 128:(lc + 1) * 128],
                                 convLF[:, lc, fc * 128:(fc + 1) * 128], ident_bf[:],
                                 is_transpose=True, start=(lc == 0), stop=(lc == 3))
            nc.vector.tensor_copy(out=convFL[:, fc, :], in_=pst[:])
        E_sb = work.tile([128, 4, 512], BF16, tag="E_sb", name=f"E_{b}")
        for lc in range(4):
            psg = bigps(f"ps_G_{b}_{lc}")
            for fc in range(4):
                nc.tensor.matmul(psg[:], convFL[:, fc, lc * 128:(lc + 1) * 128],
                                 M_sb[:, fc, :], start=(fc == 0), stop=(fc == 3))
            nc.scalar.activation(out=E_sb[:, lc, :], in_=psg[:], func=AF.Exp)
        # pre-sum the four l-chunks of E on DVE so the partition-sum needs a
        # single N=512 matmul stream instead of four (f32r: full PE rate, and
        # its ~tf32 rounding is negligible for a 512-term positive sum)
        E_sum = work.tile([128, 512], F32R, tag="E_sum", name=f"E_sum_{b}")
        nc.vector.tensor_tensor(E_sum[:], E_sb[:, 0, :], E_sb[:, 1, :], ALU.add)
        nc.vector.tensor_tensor(E_sum[:], E_sum[:], E_sb[:, 2, :], ALU.add)
        nc.vector.tensor_tensor(E_sum[:], E_sum[:], E_sb[:, 3, :], ALU.add)
        ps_s = ps_sm.tile([1, 512], F32, tag="sm", name=f"ps_s_{b}")
        nc.tensor.matmul(ps_s[:], ones_r[:], E_sum[:], start=True, stop=True)
        s_row = work.tile([1, 512], F32, tag="s_row_b", name=f"s_row_b{b}")
        nc.vector.tensor_copy(out=s_row[:], in_=ps_s[:])
        ps_rT = ps_sm.tile([128, 4], F32, tag="sm", name=f"ps_rT_{b}")
        for rc in range(4):
            nc.tensor.matmul(ps_rT[:, rc:rc + 1], s_row[:, rc * 128:(rc + 1) * 128],
                             ident[:1, :1], is_transpose=True,
                             start=(rc == 0), stop=(rc == 3))
        # reciprocal on 128 partitions x 4 elems (on a (1,512) row it runs
        # on a single DVE lane and costs ~3.3us)
        rT = work.tile([128, 4], F32, tag="rT", name=f"rT_{b}")
        nc.vector.reciprocal(out=rT[:], in_=ps_rT[:])
        woT = wo_pool.tile([128, 4, 512], BF16, tag="woT", name=f"woT_{b}")
        for rc in range(4):
            psw = bigps(f"ps_wo_{b}_{rc}")
            for lc in range(4):
                nc.tensor.matmul(psw[:], E_sb[:, lc, rc * 128:(rc + 1) * 128],
                                 convLF[:, lc, :], start=(lc == 0), stop=(lc == 3))
            if rc % 2 == 0:
                nc.vector.tensor_scalar_mul(woT[:, rc, :], psw[:], rT[:, rc:rc + 1])
            else:
                nc.scalar.activation(out=woT[:, rc, :], in_=psw[:], func=AF.Copy,
                                     scale=rT[:, rc:rc + 1])
        woTs[b] = woT

    def tail(b):
        woT = woTs.pop(b)
        Q = work.tile([128, 512], BF16, tag="Q", name=f"Q_{b}")
        nc.vector.tensor_tensor(Q[:], woT[:, 0, :], woT[:, 1, :], ALU.max)
        nc.vector.tensor_tensor(Q[:], Q[:], woT[:, 2, :], ALU.max)
        nc.vector.tensor_tensor(Q[:], Q[:], woT[:, 3, :], ALU.max)
        psq = ps_bf.tile([128, 512], BF16, tag="bigbf", name=f"ps_qT_{b}")
        for fc in range(4):
            nc.tensor.matmul(psq[:, fc * 128:(fc + 1) * 128], Q[:, fc * 128:(fc + 1) * 128],
                             ident_bf[:], is_transpose=True, start=(fc == 0), stop=(fc == 3))
        for fc in range(4):
            nc.vector.reduce_max(out=out_acc[:, b, fc:fc + 1],
                                 in_=psq[:, fc * 128:(fc + 1) * 128], axis=AX.X)

    alloc_A(0)
    for i in range(GB):
        phaseA_batch(i, 0, i)
        if i > 0:
            phaseA_mm(0, i - 1)
    phaseA_mm(0, GB - 1)
    alpha_group(0)
    for q in range(NG):
        if q + 1 < NG:
            alloc_A(q + 1)
        for i in range(GB):
            b = q * GB + i
            if q + 1 < NG:
                phaseA_batch((q + 1) * GB + i, q + 1, i)
                if i > 0:
                    phaseA_mm(q + 1, i - 1)
            head(b)
            if b > 0:
                tail(b - 1)
        if q + 1 < NG:
            phaseA_mm(q + 1, GB - 1)
            alpha_group(q + 1)
    tail(BS - 1)

    # ---- epilogue: relu + transpose to (BS, NF), store ----
    pso = ps_sm.tile([BS, NF], F32, tag="sm", name="ps_out")
    for fc in range(4):
        nc.tensor.matmul(pso[:, fc * 128:(fc + 1) * 128], out_acc[:, :, fc], ident[:],
                         is_transpose=True, start=(fc == 0), stop=(fc == 3))
    nc.scalar.activation(out=outT[:], in_=pso[:], func=AF.Relu)
    nc.sync.dma_start(out=out_ext, in_=outT[:])
    ctx.close()


_CACHE = {}


def _get_graph(has_bias: bool):
    if has_bias not in _CACHE:
        _CACHE[has_bias] = _build_graph(has_bias)
    return _CACHE[has_bias]


def kernel(**inputs) -> tuple:
    inputs = {k: np.ascontiguousarray(np.asarray(v, dtype=np.float32))
              for k, v in inputs.items()}
    has_bias = bool(np.any(inputs["conv_b"]))
    nc = _get_graph(has_bias)

    in_maps = []
    for c in range(NCORES):
        sl = slice(c * BS, (c + 1) * BS)
        in_maps.append({
            "x": inputs["x"][sl],
            "e1": inputs["e1"][sl],
            "e2": inputs["e2"][sl],
            "posVec": inputs["posVec"][sl],
            "We1": inputs["We1"],
            "We2": inputs["We2"],
            "U": inputs["U"],
            "conv_w": inputs["conv_w"],
            "conv_b": inputs["conv_b"],
            "rel_w": inputs["rel_w"],
        })
    res = run_bass_kernel_spmd(nc, in_maps, core_ids=list(range(NCORES)))
    wo = np.concatenate([res.results[c]["out"] for c in range(NCORES)], axis=0)
    return wo, inputs["rel_w"]


if __name__ == "__main__":
    rng = np.random.default_rng(0)
    ins = {
        "x": rng.standard_normal((B, L, DW), dtype=np.float32),
        "e1": rng.standard_normal((B, DW), dtype=np.float32),
        "e2": rng.standard_normal((B, DW), dtype=np.float32),
        "posVec": rng.standard_normal((B, L, 2 * DP), dtype=np.float32),
        "We1": rng.standard_normal((DW, DW), dtype=np.float32),
        "We2": rng.standard_normal((DW, DW), dtype=np.float32),
        "U": rng.standard_normal((NF, NF), dtype=np.float32),
        "conv_w": (rng.standard_normal((NF, 3 * D)) * 0.02).astype(np.float32),
        "conv_b": np.zeros(NF, np.float32),
        "rel_w": (rng.standard_normal((NF, NF)) * 0.02).astype(np.float32),
    }
    out, _ = kernel(**ins)
    print("kernel ran, out shape", out.shape)


# revision 59
# speedup vs baseline: 1.1864x; 1.1864x over previous
"""Trainium2 Bass kernel for nn_ACNN (attention CNN with attentive pooling).

Data-parallel over the batch dim: 128 batches -> 16 per NeuronCore x 8 cores.
Each core runs an identical single-core graph on its shard; no collectives.

Per-batch pipeline (all matmuls on TensorE, N=512 free dim):
  xct   = x_concat^T (d on partitions) via PE transposes
  A1/A2 = x @ (We@e) accumulated for all 16 batches into one (16,512) PSUM
          via block-diagonal lhsT (full fp32: softmax over ~N(0,256) logits
          amplifies matmul rounding, bf16-grade precision is NOT enough here)
  alpha = 0.5*(softmax(A1)+softmax(A2))  (free-axis softmax, fused exp+sum)
  conv  = tanh(alpha[l] * windowed-conv(xc))   window shifts are free-axis AP
          offsets into the zero-padded xct; alpha applied as per-partition
          ACT scale on PSUM eviction (float32r matmuls)
  G     = conv @ (U @ rel_w^T)      E = exp(G)   (G in [-14,14]: no max sub)
  s     = ones^T @ E (column sums), AP = E/s folded into the wo eviction
  wo    = relu(max_r (E^T @ conv)[r,f] / s[r])
"""

import sys

sys.path.insert(0, "/opt/trn_rl_repo")

import numpy as np

import concourse.bass as bass
import concourse.bacc as bacc
import concourse.mybir as mybir
import concourse.tile as tile
from concourse.bass_utils import run_bass_kernel_spmd
from concourse.masks import make_identity

B, L, DW, DP, NF = 128, 512, 256, 64, 512
D = DW + 2 * DP            # 384
NCORES = 8
BS = B // NCORES           # 16 batches per core
F32 = mybir.dt.float32
F32R = mybir.dt.float32r
BF16 = mybir.dt.bfloat16
FP16 = mybir.dt.float16
AF = mybir.ActivationFunctionType
ALU = mybir.AluOpType
AX = mybir.AxisListType


def r32(ap):
    return ap.bitcast(F32R)


def _build_graph(has_bias: bool):
    nc = bacc.Bacc("TRN2", target_bir_lowering=False, debug=False)

    x_ext = nc.dram_tensor("x", [BS, L, DW], F32, kind="ExternalInput").ap()
    e1_ext = nc.dram_tensor("e1", [BS, DW], F32, kind="ExternalInput").ap()
    e2_ext = nc.dram_tensor("e2", [BS, DW], F32, kind="ExternalInput").ap()
    pv_ext = nc.dram_tensor("posVec", [BS, L, 2 * DP], F32, kind="ExternalInput").ap()
    We1_ext = nc.dram_tensor("We1", [DW, DW], F32, kind="ExternalInput").ap()
    We2_ext = nc.dram_tensor("We2", [DW, DW], F32, kind="ExternalInput").ap()
    U_ext = nc.dram_tensor("U", [NF, NF], F32, kind="ExternalInput").ap()
    cw_ext = nc.dram_tensor("conv_w", [NF, 3 * D], F32, kind="ExternalInput").ap()
    cb_ext = nc.dram_tensor("conv_b", [NF], F32, kind="ExternalInput").ap()
    rw_ext = nc.dram_tensor("rel_w", [NF, NF], F32, kind="ExternalInput").ap()
    out_ext = nc.dram_tensor("out", [BS, NF], F32, kind="ExternalOutput").ap()

    with tile.TileContext(nc) as tc:
        _body(nc, tc, has_bias,
              x_ext, e1_ext, e2_ext, pv_ext, We1_ext, We2_ext, U_ext,
              cw_ext, cb_ext, rw_ext, out_ext)
    nc.compile()
    return nc


def _body(nc, tc, has_bias, x_ext, e1_ext, e2_ext, pv_ext, We1_ext, We2_ext,
          U_ext, cw_ext, cb_ext, rw_ext, out_ext):
    from contextlib import ExitStack

    GB = 4           # batches per pipeline group
    NG = BS // GB
    ctx = ExitStack()
    const = ctx.enter_context(tc.tile_pool(name="const", bufs=1))
    ps_big = ctx.enter_context(tc.tile_pool(name="ps_big", bufs=4, space="PSUM"))
    ps_bf = ctx.enter_context(tc.tile_pool(name="ps_bf", bufs=2, space="PSUM"))
    ps_sm = ctx.enter_context(tc.tile_pool(name="ps_sm", bufs=1, space="PSUM"))

    def bigps(name, shape=(128, 512), dtype=F32):
        return ps_big.tile(list(shape), dtype, tag="big", name=name)

    # ---- constants ----
    ident = const.tile([128, 128], F32, tag="ident", name="ident")
    make_identity(nc, ident[:])
    ident_bf = const.tile([128, 128], BF16, tag="ident_bf", name="ident_bf")
    nc.vector.tensor_copy(out=ident_bf[:], in_=ident[:])
    ones_bf = const.tile([128, 1], BF16, tag="ones_bf", name="ones_bf")
    nc.gpsimd.memset(ones_bf[:], 1.0)
    ones_r = const.tile([128, 1], F32R, tag="ones_r", name="ones_r")
    nc.vector.tensor_copy(out=ones_r[:], in_=ones_bf[:])
    # mask[p, b, b'] = 1 if b == b' else 0  (for block-diagonal V build)
    mask = const.tile([128, BS, BS], F32, tag="mask", name="mask")
    nc.gpsimd.memset(mask[:], 0.0)
    nc.gpsimd.affine_select(
        out=mask[:], in_=mask[:], compare_op=ALU.not_equal, fill=1.0,
        base=0, pattern=[[1, BS], [-1, BS]], channel_multiplier=0)

    convwT = const.tile([128, 9, 512], BF16, tag="convwT", name="convwT")
    M_sb = const.tile([128, 4, 512], BF16, tag="M_sb", name="M_sb")
    # The A-path stays full fp32: softmax over ~N(0,256) logits amplifies
    # input rounding (bf16 -> ~0.5 max-abs output error, fp16 -> ~4e-2);
    # fp32 keeps max-abs error at ~5e-3. V1 and V2 are stacked along the
    # output-row (M) axis so one N=512 stream computes both A1 and A2.
    # dims: [p, b, dc, group, v, within-group] so the per-(b,dc,group) lhsT
    # slice is one contiguous 2*GB-wide free range (matmul weights APs must
    # flatten to a single free dim)
    VblkC = const.tile([128, BS, 2, NG, 2, GB], F32, tag="VblkC", name="VblkC")
    alpha_colT = const.tile([128, 4, BS], F32, tag="alphaT", name="alphaT")
    out_acc = const.tile([128, BS, 4], F32, tag="out_acc", name="out_acc")
    outT = const.tile([BS, NF], F32, tag="outT", name="outT")
    if has_bias:
        b_bcast = const.tile([128, 512], F32, tag="b_bcast", name="b_bcast")

    # ---- setup: transpose weights, precompute M = U @ rel_w^T, V1/V2 ----
    # DMA issues go smallest-first so the PE's first work (the We/e
    # transposes) needs only ~0.5MB of loads instead of waiting ~12us for
    # the 2.25MB conv_w tensor; conv_w isn't consumed until phase B.
    with tc.tile_pool(name="setup", bufs=1) as setup:
        st_ws, st_es = {}, {}
        for name, W_ext, e_ext in (("1", We1_ext, e1_ext), ("2", We2_ext, e2_ext)):
            st_w = setup.tile([128, 2, 256], F32, tag=f"st_w{name}", name=f"st_w{name}")
            nc.sync.dma_start(out=st_w[:], in_=W_ext.rearrange("(dc p) e -> p dc e", p=128))
            st_e = setup.tile([BS, 256], F32, tag=f"st_e{name}", name=f"st_e{name}")
            nc.sync.dma_start(out=st_e[:], in_=e_ext)
            st_ws[name], st_es[name] = st_w, st_e
        st_u = setup.tile([128, 4, 512], F32, tag="st_u", name="st_u")
        nc.sync.dma_start(out=st_u[:], in_=U_ext.rearrange("(fc p) t -> p fc t", p=128))
        st_rw = setup.tile([128, 4, 512], F32, tag="st_rw", name="st_rw")
        nc.sync.dma_start(out=st_rw[:], in_=rw_ext.rearrange("(rc p) t -> p rc t", p=128))
        st_cw = setup.tile([128, 4, 3 * D], F32, tag="st_cw", name="st_cw")
        nc.sync.dma_start(out=st_cw[:], in_=cw_ext.rearrange("(fc p) kd -> p fc kd", p=128))

        # We1/We2 (256d, 256e) -> WeT (e part, d free); e1/e2 -> eT (e part, b free)
        V_sb = {}
        for name in ("1", "2"):
            st_w, st_e = st_ws[name], st_es[name]
            WT = setup.tile([128, 2, 256], F32, tag=f"WT{name}", name=f"WT{name}")
            for ec in range(2):
                psw = bigps(f"ps_WT{name}_{ec}", (128, 256))
                for dc in range(2):
                    nc.tensor.matmul(psw[:, dc * 128:(dc + 1) * 128],
                                     st_w[:, dc, ec * 128:(ec + 1) * 128], ident[:],
                                     is_transpose=True, start=(dc == 0), stop=(dc == 1))
                nc.vector.tensor_copy(out=WT[:, ec, :], in_=psw[:])
            pse = ps_sm.tile([128, 2 * BS], F32, tag="sm", name=f"ps_eT{name}")
            for ec in range(2):
                nc.tensor.matmul(pse[:, ec * BS:(ec + 1) * BS],
                                 st_e[:, ec * 128:(ec + 1) * 128], ident[:BS, :BS],
                                 is_transpose=True, start=(ec == 0), stop=(ec == 1))
            eT = setup.tile([128, 2, BS], F32, tag=f"eT{name}", name=f"eT{name}")
            nc.vector.tensor_copy(out=eT[:], in_=pse[:])
            # V[d, b] = sum_e We[d, e] e[b, e]   (full fp32 - feeds the A softmax)
            V = setup.tile([128, 2, BS], F32, tag=f"V{name}", name=f"V{name}")
            for dc in range(2):
                psv = ps_sm.tile([128, BS], F32, tag="sm", name=f"ps_V{name}_{dc}")
                for ec in range(2):
                    nc.tensor.matmul(psv[:], WT[:, ec, dc * 128:(dc + 1) * 128],
                                     eT[:, ec, :], start=(ec == 0), stop=(ec == 1))
                nc.vector.tensor_copy(out=V[:, dc, :], in_=psv[:])
            V_sb[name] = V

        # VblkC[p, b, dc, q, v, g] = V_v[p, dc, b] * (b == q*GB+g)
        # (one TT op per (v, group): the ISA allows at most 3 free dims)
        for v, name in ((0, "1"), (1, "2")):
            for q in range(NG):
                nc.vector.tensor_tensor(
                    VblkC[:, :, :, q, v, :],
                    V_sb[name].rearrange("p dc b -> p b dc")[:, :, :, None]
                    .to_broadcast([128, BS, 2, GB]),
                    mask[:, :, None, q * GB:(q + 1) * GB]
                    .to_broadcast([128, BS, 2, GB]),
                    ALU.mult)

        # U (512f, 512t) -> UT (t part, f free); rel_w (512r, 512t) -> relwT
        UT = setup.tile([128, 4, 512], F32R, tag="UT", name="UT")
        relwT = setup.tile([128, 4, 512], F32R, tag="relwT", name="relwT")
        for tc_i in range(4):
            psu = bigps(f"ps_UT_{tc_i}")
            for fc in range(4):
                nc.tensor.matmul(psu[:, fc * 128:(fc + 1) * 128],
                                 st_u[:, fc, tc_i * 128:(tc_i + 1) * 128], ident[:],
                                 is_transpose=True, start=(fc == 0), stop=(fc == 3))
            nc.vector.tensor_copy(out=UT[:, tc_i, :], in_=psu[:])
            psr = bigps(f"ps_rwT_{tc_i}")
            for rc in range(4):
                nc.tensor.matmul(psr[:, rc * 128:(rc + 1) * 128],
                                 st_rw[:, rc, tc_i * 128:(tc_i + 1) * 128], ident[:],
                                 is_transpose=True, start=(rc == 0), stop=(rc == 3))
            nc.vector.tensor_copy(out=relwT[:, tc_i, :], in_=psr[:])
        # M[f, r] = sum_t U[f,t] rel_w[r,t]
        for fc in range(4):
            psm = bigps(f"ps_M_{fc}")
            for tc_i in range(4):
                nc.tensor.matmul(psm[:], UT[:, tc_i, fc * 128:(fc + 1) * 128],
                                 relwT[:, tc_i, :],
                                 start=(tc_i == 0), stop=(tc_i == 3))
            nc.vector.tensor_copy(out=M_sb[:, fc, :], in_=psm[:])

        # conv_w (512f, 1152kd) -> convwT[kd-part chunks, f free]
        # (last: its 2.25MB load is the slowest, and phase B is its consumer)
        for kd in range(9):
            ps = bigps(f"ps_cwT_{kd}")
            for fc in range(4):
                nc.tensor.matmul(ps[:, fc * 128:(fc + 1) * 128],
                                 st_cw[:, fc, kd * 128:(kd + 1) * 128], ident[:],
                                 is_transpose=True, start=(fc == 0), stop=(fc == 3))
            nc.vector.tensor_copy(out=convwT[:, kd, :], in_=ps[:])

        if has_bias:
            st_b = setup.tile([1, 512], F32, tag="st_b", name="st_b")
            nc.sync.dma_start(out=st_b[:], in_=cb_ext[None, :])
            ones_row = setup.tile([1, 128], F32, tag="ones_row", name="ones_row")
            nc.gpsimd.memset(ones_row[:], 1.0)
            psb = bigps("ps_bb")
            nc.tensor.matmul(psb[:], ones_row[:], st_b[:], start=True, stop=True)
            nc.vector.tensor_copy(out=b_bcast[:], in_=psb[:])

    # ---- main pools ----
    xct_pool = ctx.enter_context(tc.tile_pool(name="xct", bufs=BS))
    work = ctx.enter_context(tc.tile_pool(name="work", bufs=3))
    wo_pool = ctx.enter_context(tc.tile_pool(name="wo", bufs=2))
    stage = ctx.enter_context(tc.tile_pool(name="stage", bufs=3))

    # Batches are processed in NG groups of GB. Group q's loads/transposes/
    # attention logits (phase A) are emitted interleaved with group q-1's
    # conv/pool work (phase B) so the PE never drains while DMA/DVE chains
    # mature. alpha (the batch-softmax barrier) is per group.
    xcts = [None] * BS
    A_pss = {}
    pending_A = {}

    def phaseA_batch(b, q, i):
        x_st = stage.tile([128, 4, DW], F32, tag="x_st", name=f"x_st_{b}")
        nc.sync.dma_start(out=x_st[:], in_=x_ext[b].rearrange("(lc p) d -> p lc d", p=128))
        pv_st = stage.tile([128, 4, 2 * DP], F32, tag="pv_st", name=f"pv_st_{b}")
        nc.sync.dma_start(out=pv_st[:], in_=pv_ext[b].rearrange("(lc p) d -> p lc d", p=128))
        xct = xct_pool.tile([128, 3, L + 2], BF16, tag="xct", name=f"xct_{b}")
        nc.gpsimd.memset(xct[:, :, 0:1], 0.0)
        nc.gpsimd.memset(xct[:, :, L + 1:L + 2], 0.0)
        A_ps = A_pss[q]
        xtmps = []
        for dd in range(3):
            ps = bigps(f"ps_xT_{b}_{dd}")
            for lc in range(4):
                src = (x_st[:, lc, dd * 128:(dd + 1) * 128] if dd < 2
                       else pv_st[:, lc, :])
                nc.tensor.matmul(ps[:, lc * 128:(lc + 1) * 128], src, ident[:],
                                 is_transpose=True, start=(lc == 0), stop=(lc == 3))
            nc.vector.tensor_copy(out=xct[:, dd, 1:L + 1], in_=ps[:])
            if dd < 2:
                # full-fp32 copy for the A-path (bf16 rounding of x would
                # perturb the ~N(0,256) attention logits too much)
                xtmp = stage.tile([128, 512], F32, tag="xtmp", bufs=4,
                                  name=f"xtmp_{b}_{dd}")
                nc.scalar.activation(out=xtmp[:], in_=ps[:], func=AF.Copy)
                xtmps.append(xtmp)
        xcts[b] = xct
        pending_A[(q, i)] = xtmps

    def phaseA_mm(q, i):
        # Deferred one interleave step so the xtmp ACT copies (queued behind
        # the previous batch's tanh/exp on the in-order Scalar engine) have
        # matured before the in-order PE reaches these matmuls. One matmul
        # computes both A1 (rows 0..GB-1) and A2 (rows GB..2GB-1).
        b = q * GB + i
        xtmps = pending_A.pop((q, i))
        A_ps = A_pss[q]
        for dd in range(2):
            nc.tensor.matmul(A_ps[:], VblkC[:, b, dd, q, :, :],
                             xtmps[dd][:], start=(i == 0 and dd == 0),
                             stop=(i == GB - 1 and dd == 1))

    def alpha_group(q):
        # softmax over both stacked logit halves in one pass; the
        # alpha = 0.5*(softmax(A1)+softmax(A2)) combine happens after the
        # transpose, where the halves sit GB columns apart (a free-axis
        # add -- DVE cannot add across partitions).
        A_ps = A_pss.pop(q)
        neg_m = work.tile([2 * GB, 1], F32, tag="neg_m", name=f"neg_m_{q}")
        nc.vector.reduce_max(out=neg_m[:], in_=A_ps[:], axis=AX.X, negate=True)
        E_row = work.tile([2 * GB, 512], F32, tag="E_row", name=f"E_row_{q}")
        s_row = work.tile([2 * GB, 1], F32, tag="s_row", name=f"s_row_{q}")
        nc.scalar.activation(out=E_row[:], in_=A_ps[:], func=AF.Exp,
                             bias=neg_m[:], scale=1.0, accum_out=s_row[:])
        r_row = work.tile([2 * GB, 1], F32, tag="r_row", name=f"r_row_{q}")
        nc.vector.reciprocal(out=r_row[:], in_=s_row[:])
        nc.vector.tensor_scalar_mul(r_row[:], r_row[:], 0.5)
        nc.vector.tensor_scalar_mul(E_row[:], E_row[:], r_row[:])
        psa = ps_sm.tile([128, 4, 2 * GB], F32, tag="sm", name=f"ps_alphaT_{q}")
        for lc in range(4):
            nc.tensor.matmul(psa[:, lc, :], E_row[:, lc * 128:(lc + 1) * 128],
                             ident[:2 * GB, :2 * GB],
                             is_transpose=True, start=(lc == 0), stop=(lc == 3))
        # TT ops may read only one PSUM operand: evict, then add the halves
        aT = work.tile([128, 4, 2 * GB], F32, tag="aT", name=f"aT_{q}")
        nc.vector.tensor_copy(out=aT[:], in_=psa[:])
        nc.vector.tensor_tensor(
            alpha_colT[:, :, q * GB:(q + 1) * GB],
            aT[:, :, 0:GB], aT[:, :, GB:2 * GB], ALU.add)

    def alloc_A(q):
        # bufs=1: group q's tile is read+released by alpha(q) before group
        # q+1's first A-matmul executes, so one slot never conflicts
        A_pss[q] = ps_sm.tile([2 * GB, 512], F32, tag="Aps", bufs=1,
                              name=f"A_ps_{q}")

    # ---- phase B: conv -> G -> softmax-pool -> wo ----
    # The per-batch tail (max over r + transpose + reduce) is a serial
    # DVE chain; emit it one batch late so the PE fills the wait with the
    # next batch's conv matmuls (also keeps HAM un-throttled).
    woTs = {}

    def head(b):
        xct = xcts[b]
        convLF = work.tile([128, 4, 512], BF16, tag="convLF", name=f"convLF_{b}")
        for lc in range(4):
            psc = bigps(f"ps_conv_{b}_{lc}")
            first = True
            for k in range(3):
                for dc in range(3):
                    nc.tensor.matmul(
                        psc[:],
                        xct[:, dc, lc * 128 + k: lc * 128 + k + 128],
                        convwT[:, k * 3 + dc, :],
                        start=first, stop=(k == 2 and dc == 2))
                    first = False
            if has_bias:
                t_sc = work.tile([128, 512], F32, tag="t_sc", name=f"t_sc_{b}_{lc}")
                nc.vector.tensor_scalar_mul(t_sc[:], psc[:], alpha_colT[:, lc, b:b + 1])
                nc.vector.tensor_tensor(t_sc[:], t_sc[:], b_bcast[:], ALU.add)
                nc.scalar.activation(out=convLF[:, lc, :], in_=t_sc[:], func=AF.Tanh)
            else:
                nc.scalar.activation(out=convLF[:, lc, :], in_=psc[:], func=AF.Tanh,
                                     bias=0.0, scale=alpha_colT[:, lc, b:b + 1])
        convFL = work.tile([128, 4, 512], BF16, tag="convFL", name=f"convFL_{b}")
        for fc in range(4):
            pst = ps_bf.tile([128, 512], BF16, tag="bigbf", name=f"ps_cT_{b}_{fc}")
            for lc in range(4):
                nc.tensor.matmul(pst[:, lc * 128:(lc + 1) * 128],
                                 convLF[:, lc, fc * 128:(fc + 1) * 128], ident_bf[:],
                                 is_transpose=True, start=(lc == 0), stop=(lc == 3))
            nc.vector.tensor_copy(out=convFL[:, fc, :], in_=pst[:])
        E_sb = work.tile([128, 4, 512], BF16, tag="E_sb", name=f"E_{b}")
        for lc in range(4):
            psg = bigps(f"ps_G_{b}_{lc}")
            for fc in range(4):
                nc.tensor.matmul(psg[:], convFL[:, fc, lc * 128:(lc + 1) * 128],
                                 M_sb[:, fc, :], start=(fc == 0), stop=(fc == 3))
            nc.scalar.activation(out=E_sb[:, lc, :], in_=psg[:], func=AF.Exp)
        # pre-sum the four l-chunks of E on DVE so the partition-sum needs a
        # single N=512 matmul stream instead of four (f32r: full PE rate, and
        # its ~tf32 rounding is negligible for a 512-term positive sum)
        E_sum = work.tile([128, 512], F32R, tag="E_sum", name=f"E_sum_{b}")
        nc.vector.tensor_tensor(E_sum[:], E_sb[:, 0, :], E_sb[:, 1, :], ALU.add)
        nc.vector.tensor_tensor(E_sum[:], E_sum[:], E_sb[:, 2, :], ALU.add)
        nc.vector.tensor_tensor(E_sum[:], E_sum[:], E_sb[:, 3, :], ALU.add)
        ps_s = ps_sm.tile([1, 512], F32, tag="sm", name=f"ps_s_{b}")
        nc.tensor.matmul(ps_s[:], ones_r[:], E_sum[:], start=True, stop=True)
        s_row = work.tile([1, 512], F32, tag="s_row_b", name=f"s_row_b{b}")
        nc.vector.tensor_copy(out=s_row[:], in_=ps_s[:])
        ps_rT = ps_sm.tile([128, 4], F32, tag="sm", name=f"ps_rT_{b}")
        for rc in range(4):
            nc.tensor.matmul(ps_rT[:, rc:rc + 1], s_row[:, rc * 128:(rc + 1) * 128],
                             ident[:1, :1], is_transpose=True,
                             start=(rc == 0), stop=(rc == 3))
        # reciprocal on 128 partitions x 4 elems (on a (1,512) row it runs
        # on a single DVE lane and costs ~3.3us)
        rT = work.tile([128, 4], F32, tag="rT", name=f"rT_{b}")
        nc.vector.reciprocal(out=rT[:], in_=ps_rT[:])
        woT = wo_pool.tile([128, 4, 512], BF16, tag="woT", name=f"woT_{b}")
        for rc in range(4):
            psw = bigps(f"ps_wo_{b}_{rc}")
            for lc in range(4):
                nc.tensor.matmul(psw[:], E_sb[:, lc, rc * 128:(rc + 1) * 128],
                                 convLF[:, lc, :], start=(lc == 0), stop=(lc == 3))
            if rc % 2 == 0:
                nc.vector.tensor_scalar_mul(woT[:, rc, :], psw[:], rT[:, rc:rc + 1])
            else:
                nc.scalar.activation(out=woT[:, rc, :], in_=psw[:], func=AF.Copy,
                                     scale=rT[:, rc:rc + 1])
        woTs[b] = woT

    def tail(b):
        woT = woTs.pop(b)
        Q = work.tile([128, 512], BF16, tag="Q", name=f"Q_{b}")
        nc.vector.tensor_tensor(Q[:], woT[:, 0, :], woT[:, 1, :], ALU.max)
        nc.vector.tensor_tensor(Q[:], Q[:], woT[:, 2, :], ALU.max)
        nc.vector.tensor_tensor(Q[:], Q[:], woT[:, 3, :], ALU.max)
        psq = ps_bf.tile([128, 512], BF16, tag="bigbf", name=f"ps_qT_{b}")
        for fc in range(4):
            nc.tensor.matmul(psq[:, fc * 128:(fc + 1) * 128], Q[:, fc * 128:(fc + 1) * 128],
                             ident_bf[:], is_transpose=True, start=(fc == 0), stop=(fc == 3))
        for fc in range(4):
            nc.vector.reduce_max(out=out_acc[:, b, fc:fc + 1],
                                 in_=psq[:, fc * 128:(fc + 1) * 128], axis=AX.X)

    alloc_A(0)
    for i in range(GB):
        phaseA_batch(i, 0, i)
        if i > 0:
            phaseA_mm(0, i - 1)
    phaseA_mm(0, GB - 1)
    alpha_group(0)
    for q in range(NG):
        if q + 1 < NG:
            alloc_A(q + 1)
        for i in range(GB):
            b = q * GB + i
            if q + 1 < NG:
                phaseA_batch((q + 1) * GB + i, q + 1, i)
                if i > 0:
                    phaseA_mm(q + 1, i - 1)
            head(b)
            if b > 0:
                tail(b - 1)
        if q + 1 < NG:
            phaseA_mm(q + 1, GB - 1)
            alpha_group(q + 1)
    tail(BS - 1)

    # ---- epilogue: relu + transpose to (BS, NF), store ----
    pso = ps_sm.tile([BS, NF], F32, tag="sm", name="ps_out")
    for fc in range(4):
        nc.tensor.matmul(pso[:, fc * 128:(fc + 1) * 128], out_acc[:, :, fc], ident[:],
                         is_transpose=True, start=(fc == 0), stop=(fc == 3))
    nc.scalar.activation(out=outT[:], in_=pso[:], func=AF.Relu)
    nc.sync.dma_start(out=out_ext, in_=outT[:])
    ctx.close()


_CACHE = {}


def _get_graph(has_bias: bool):
    if has_bias not in _CACHE:
        _CACHE[has_bias] = _build_graph(has_bias)
    return _CACHE[has_bias]


def kernel(**inputs) -> tuple:
    inputs = {k: np.ascontiguousarray(np.asarray(v, dtype=np.float32))
              for k, v in inputs.items()}
    has_bias = bool(np.any(inputs["conv_b"]))
    nc = _get_graph(has_bias)

    in_maps = []
    for c in range(NCORES):
        sl = slice(c * BS, (c + 1) * BS)
        in_maps.append({
            "x": inputs["x"][sl],
            "e1": inputs["e1"][sl],
            "e2": inputs["e2"][sl],
            "posVec": inputs["posVec"][sl],
            "We1": inputs["We1"],
            "We2": inputs["We2"],
            "U": inputs["U"],
            "conv_w": inputs["conv_w"],
            "conv_b": inputs["conv_b"],
            "rel_w": inputs["rel_w"],
        })
    res = run_bass_kernel_spmd(nc, in_maps, core_ids=list(range(NCORES)))
    wo = np.concatenate([res.results[c]["out"] for c in range(NCORES)], axis=0)
    return wo, inputs["rel_w"]


if __name__ == "__main__":
    rng = np.random.default_rng(0)
    ins = {
        "x": rng.standard_normal((B, L, DW), dtype=np.float32),
        "e1": rng.standard_normal((B, DW), dtype=np.float32),
        "e2": rng.standard_normal((B, DW), dtype=np.float32),
        "posVec": rng.standard_normal((B, L, 2 * DP), dtype=np.float32),
        "We1": rng.standard_normal((DW, DW), dtype=np.float32),
        "We2": rng.standard_normal((DW, DW), dtype=np.float32),
        "U": rng.standard_normal((NF, NF), dtype=np.float32),
        "conv_w": (rng.standard_normal((NF, 3 * D)) * 0.02).astype(np.float32),
        "conv_b": np.zeros(NF, np.float32),
        "rel_w": (rng.standard_normal((NF, NF)) * 0.02).astype(np.float32),
    }
    out, _ = kernel(**ins)
    print("kernel ran, out shape", out.shape)
